# revision 1
# baseline (speedup 1.0000x reference)
"""Bayesian linear layer on 8 Trainium2 NeuronCores.

Computes: weight = mu + softplus(rho) * eps  (elementwise, [O, I])
          bias   = b_mu + softplus(b_rho) * b_eps              ([O])
          y      = x @ weight.T + bias       ([N, I] @ [I, O] -> [N, O])

Shapes: x [8192, 4096], weight_* [16384, 4096], bias_* [16384].

Sharding: column-parallel over 8 cores — each core owns 2048 output
features (its slice of the weight/bias params), x is replicated. Each
core computes an independent [8192, 2048] output slice; the host
concatenates along the feature dim. No collectives needed.

Device kernel (SPMD, one Bass program, per-core data):
 - softplus computed as Ln(Exp(rho) + 1) on the scalar engine (the
   container's act tables lack a direct softplus entry).
 - weights materialized on-chip into resident bf16 SBUF tiles
   [128 i-part, 2048 o] x 32 k-tiles (128 KB/partition).
 - x streamed as transposed bf16 tiles [128 i-part, 32 kt, 128 n];
   host pre-transposes x (both matmul operands need the contraction
   dim on partitions; DMA transpose only supports 2-byte dtypes and
   strided f32 gathers are far off line-rate).
 - matmul: out[n, o] += xT_tile.T @ w_tile, PSUM [128 n, 2048 o]
   (4 banks), 32-step K accumulation, bias added during the PSUM->SBUF
   copy (one DVE pass), then DMA to DRAM.
"""

import numpy as np
import ml_dtypes

import concourse.bass as bass
import concourse.mybir as mybir
import concourse.tile as tile
from concourse.bass_utils import run_bass_kernel_spmd
from concourse.vector_clock import ScopedClock, VectorClock

N_CORES = 8
N_TOK = 8192
IN_F = 4096
OUT_F = 16384
O_PER = OUT_F // N_CORES  # 2048 out features per core

P = 128
KT = IN_F // P       # 32 k-tiles
MT = N_TOK // P      # 64 m-tiles
OC = 512             # o-chunk for weight materialization + matmul N
NOC = O_PER // OC    # 4 o-chunks

F32 = mybir.dt.float32
BF16 = mybir.dt.bfloat16
AF = mybir.ActivationFunctionType
ALU = mybir.AluOpType


def _patch_tile_drain():
    """The walrus build here caps sync-wait commands per CTRL_NO_STRUCT
    instruction; Tile's kernel-tail Drain overflows it. Spread the waits
    across nop carriers (one wait each) before the drain."""
    if getattr(tile.TileContext, "_drain_patched", False):
        return

    def _drain_and_barrier(self, tick_clock, wait_clock):
        nc = self.nc
        gc = tick_clock.global_clock
        n = len(gc)
        for i in range(n):
            t = gc[i]
            if t > 0:
                sub = [0] * n
                sub[i] = t
                carrier = nc.sync.nop(nofuse=True)
                wait_clock.add_sem_waits(
                    carrier.ins, ScopedClock({None: VectorClock(sub)})
                )
        nc.sync.drain()
        nc.all_engine_barrier()
        popped = nc._tile_sem_poison_stack.pop()
        assert popped is self._sem_poison
        nc.clear_and_free_semaphores(list(self.sems.allocated().values()))
        nc.all_engine_barrier()

    tile.TileContext._drain_and_barrier = _drain_and_barrier
    tile.TileContext._drain_patched = True


def _split_sync_waits(nc, max_waits=1):
    """This container's walrus build accepts at most ONE sync-wait command
    per instruction (a 2-wait TensorTensor fails codegen with 'Too many
    sync wait commands'). Tile emits up to 3. Spill the excess onto
    same-engine InstNoOp carriers inserted immediately before the
    overloaded instruction — same-engine program order preserves the
    wait-before-execute semantics."""
    n_spilled = 0
    for fn in nc.m.functions:
        for bb in fn.blocks:
            insts = list(bb.instructions)
            out = []
            changed = False
            for inst in insts:
                si = inst.sync_info
                if si is not None and si.on_wait and len(si.on_wait) > max_waits:
                    waits = list(si.on_wait)
                    spill, keep = waits[:-max_waits], waits[-max_waits:]
                    for w in spill:
                        nop = mybir.InstNoOp(
                            name=f"I-waitspill-{nc.next_id()}", ins=[], outs=[]
                        )
                        nop.engine = inst.engine
                        nop.sync_info = mybir.SyncInfo(on_wait=[w], on_update=[])
                        out.append(nop)
                        n_spilled += 1
                    inst.sync_info = mybir.SyncInfo(
                        on_wait=keep, on_update=list(si.on_update)
                    )
                    changed = True
                out.append(inst)
            if changed:
                bb.instructions = out
    return n_spilled


M_CHUNK = 256            # tokens per x tile (2 lhsT subtiles of 128)
MC = N_TOK // M_CHUNK    # 32 m-chunks
MSUB = M_CHUNK // P      # 2
OCS = 512                # stage chunk for weight materialization
NSUB = OC // OCS         # 1 stage chunk per (block, k-tile)


def _build():
    """All four 512-col output blocks keep their bf16 weights resident
    (128 KB/partition). Tokens stream in two PAIR passes: pair 0 runs
    blocks {0,1} per x chunk (each x chunk feeds 1024 output cols),
    pair 1 runs blocks {2,3}. x is read twice instead of four times --
    the whole schedule is DMA-limited, so bytes are the budget. Blocks
    2/3 materialize during pair 0 into their own buffers (no WAR
    conflicts), so only blocks 0/1's params (~50 MB) gate the start."""
    _patch_tile_drain()
    nc = bass.Bass()

    xT = nc.dram_tensor("xT", [IN_F, N_TOK], BF16, kind="ExternalInput")
    wmuT = nc.dram_tensor("wmuT", [IN_F, O_PER], BF16, kind="ExternalInput")
    wrhoT = nc.dram_tensor("wrhoT", [IN_F, O_PER], BF16, kind="ExternalInput")
    wepsT = nc.dram_tensor("wepsT", [IN_F, O_PER], BF16, kind="ExternalInput")
    bmu = nc.dram_tensor("bmu", [1, O_PER], BF16, kind="ExternalInput")
    brho = nc.dram_tensor("brho", [1, O_PER], BF16, kind="ExternalInput")
    beps = nc.dram_tensor("beps", [1, O_PER], BF16, kind="ExternalInput")
    y = nc.dram_tensor("y", [N_TOK, O_PER], F32, kind="ExternalOutput")

    xT_r = xT[:, :].rearrange("(kt p) n -> p kt n", p=P)

    with tile.TileContext(nc) as tc:
        with (
            tc.tile_pool(name="wpool", bufs=1) as wpool,
            tc.tile_pool(name="stage", bufs=2) as stage,
            tc.tile_pool(name="xpool", bufs=2) as xpool,
            tc.tile_pool(name="opool", bufs=2) as opool,
            tc.tile_pool(name="bpool", bufs=1) as bpool,
            tc.tile_pool(name="psum", bufs=4, space="PSUM") as psump,
        ):
            # resident bf16 weights for all 4 o-blocks:
            # 4 x 32 x [128, 512] bf16 = 128 KB/partition
            w_tiles = {
                (j, k): wpool.tile([P, OC], BF16, name=f"w_{j}_{k}", tag=f"w_{j}_{k}")
                for j in range(NOC)
                for k in range(KT)
            }

            bias_bc = bpool.tile([P, O_PER], BF16, name="bias_bc")

            def softplus_fma(dst, rho_src, eps_src, mu_src, exp_t, sp_t, prod_t):
                # dst = mu + softplus(rho) * eps, via Ln(Exp(rho) + 1)
                nc.scalar.activation(exp_t, rho_src, AF.Exp)
                nc.scalar.activation(sp_t, exp_t, AF.Ln, bias=1.0)
                nc.vector.tensor_mul(prod_t, sp_t, eps_src)
                nc.vector.tensor_add(dst, prod_t, mu_src)

            def stage_tiles():
                rho_s = stage.tile([P, OCS], BF16, name="rho_s", tag="rho_s")
                eps_s = stage.tile([P, OCS], BF16, name="eps_s", tag="eps_s")
                mu_s = stage.tile([P, OCS], BF16, name="mu_s", tag="mu_s")
                exp_s = stage.tile([P, OCS], F32, name="exp_s", tag="exp_s")
                return rho_s, eps_s, mu_s, exp_s

            def materialize_ktile(j, k):
                # w[j, k][:, :] = mu + softplus(rho)*eps for o-block j
                ksl = slice(k * P, (k + 1) * P)
                for sub in range(NSUB):
                    csl = slice(j * OC + sub * OCS, j * OC + (sub + 1) * OCS)
                    wsl = bass.ts(sub, OCS)
                    rho_s, eps_s, mu_s, exp_s = stage_tiles()
                    sp_s = stage.tile([P, OCS], F32, name="sp_s", tag="sp_s")
                    nc.sync.dma_start(rho_s, wrhoT[ksl, csl])
                    nc.sync.dma_start(eps_s, wepsT[ksl, csl])
                    nc.sync.dma_start(mu_s, wmuT[ksl, csl])
                    softplus_fma(
                        w_tiles[(j, k)][:, wsl], rho_s, eps_s, mu_s, exp_s, sp_s, exp_s
                    )

            # ── bias: softplus fma on partition 0, then replicate to all
            # 128 partitions via a doubling SBUF->SBUF DMA ladder (the
            # InstPartitionBroadcast custom op fails codegen here).
            for oc in range(O_PER // OCS):
                sl = bass.ts(oc, OCS)
                rho_s, eps_s, mu_s, exp_s = stage_tiles()
                sp_b = stage.tile([P, OCS], F32, name="sp_s", tag="sp_s")
                nc.sync.dma_start(rho_s[0:1, :], brho[0:1, sl])
                nc.sync.dma_start(eps_s[0:1, :], beps[0:1, sl])
                nc.sync.dma_start(mu_s[0:1, :], bmu[0:1, sl])
                softplus_fma(
                    bias_bc[0:1, sl], rho_s[0:1, :], eps_s[0:1, :], mu_s[0:1, :],
                    exp_s[0:1, :], sp_b[0:1, :], exp_s[0:1, :],
                )
            rep = 1
            while rep < P:
                nc.sync.dma_start(bias_bc[rep : 2 * rep, :], bias_bc[0:rep, :])
                rep *= 2

            # ── blocks 0 and 1 up front (gate pair-0's start)
            for k in range(KT):
                materialize_ktile(0, k)
            for k in range(KT):
                materialize_ktile(1, k)

            def mm_group(xt, j, mc):
                jsl = bass.ts(j, OC)
                ps = psump.tile([P, MSUB * OC], F32, name="ps", tag="ps")
                for k in range(KT):
                    for s in range(MSUB):
                        nc.tensor.matmul(
                            ps[:, bass.ts(s, OC)],
                            xt[:, k, bass.ts(s, P)],
                            w_tiles[(j, k)],
                            start=(k == 0),
                            stop=(k == KT - 1),
                        )
                for s in range(MSUB):
                    out_sb = opool.tile([P, OC], F32, name="out_sb", tag="out_sb")
                    nc.vector.scalar_tensor_tensor(
                        out_sb,
                        ps[:, bass.ts(s, OC)],
                        1.0,
                        bias_bc[:, jsl],
                        op0=ALU.bypass,
                        op1=ALU.add,
                    )
                    nc.sync.dma_start(
                        y[mc * M_CHUNK + s * P : mc * M_CHUNK + (s + 1) * P, jsl],
                        out_sb,
                    )

            # ── pair loop: pair 0 = blocks {0,1} (blocks 2,3 materialize
            # interleaved), pair 1 = blocks {2,3}
            for pair in range(NOC // 2):
                for mc in range(MC):
                    xt = xpool.tile([P, KT, M_CHUNK], BF16, name="xt", tag="xt")
                    nc.sync.dma_start(
                        xt, xT_r[:, :, mc * M_CHUNK : (mc + 1) * M_CHUNK]
                    )
                    for dj in range(2):
                        mm_group(xt, 2 * pair + dj, mc)
                    if pair == 0:
                        materialize_ktile(2, mc)
                        materialize_ktile(3, mc)

    _split_sync_waits(nc)
    nc.finalize()
    return nc


_NC_CACHE = None


def _get_nc():
    global _NC_CACHE
    if _NC_CACHE is None:
        _NC_CACHE = _build()
    return _NC_CACHE


def prepare_in_maps(x, weight_mu, weight_rho, weight_eps, bias_mu, bias_rho, bias_eps):
    x = np.asarray(x, dtype=np.float32)
    weight_mu = np.asarray(weight_mu, dtype=np.float32)
    weight_rho = np.asarray(weight_rho, dtype=np.float32)
    weight_eps = np.asarray(weight_eps, dtype=np.float32)
    bias_mu = np.asarray(bias_mu, dtype=np.float32)
    bias_rho = np.asarray(bias_rho, dtype=np.float32)
    bias_eps = np.asarray(bias_eps, dtype=np.float32)

    xT = np.ascontiguousarray(x.T).astype(ml_dtypes.bfloat16)  # [IN_F, N_TOK]
    in_maps = []
    for c in range(N_CORES):
        osl = slice(c * O_PER, (c + 1) * O_PER)
        in_maps.append(
            {
                "xT": xT,
                "wmuT": np.ascontiguousarray(weight_mu[osl, :].T).astype(ml_dtypes.bfloat16),
                "wrhoT": np.ascontiguousarray(weight_rho[osl, :].T).astype(ml_dtypes.bfloat16),
                "wepsT": np.ascontiguousarray(weight_eps[osl, :].T).astype(ml_dtypes.bfloat16),
                "bmu": bias_mu[osl].reshape(1, O_PER).astype(ml_dtypes.bfloat16),
                "brho": bias_rho[osl].reshape(1, O_PER).astype(ml_dtypes.bfloat16),
                "beps": bias_eps[osl].reshape(1, O_PER).astype(ml_dtypes.bfloat16),
            }
        )
    return in_maps


def run(in_maps, trace=False):
    nc = _get_nc()
    res = run_bass_kernel_spmd(nc, in_maps, list(range(N_CORES)), trace=trace)
    out = np.concatenate([res.results[c]["y"] for c in range(N_CORES)], axis=1)
    return out, res


def kernel(**inputs) -> np.ndarray:
    in_maps = prepare_in_maps(**inputs)
    out, _ = run(in_maps, trace=False)
    return out



# revision 4
# speedup vs baseline: 1.0452x; 1.0452x over previous
"""Bayesian linear layer on 8 Trainium2 NeuronCores — Strassen edition.

Computes: weight = mu + softplus(rho) * eps  (elementwise, [O, I])
          bias   = b_mu + softplus(b_rho) * b_eps              ([O])
          y      = x @ weight.T + bias       ([N, I] @ [I, O] -> [N, O])

Shapes: x [8192, 4096], weight_* [16384, 4096], bias_* [16384].

Sharding: column-parallel over 8 cores — each core owns 2048 output
features, x replicated; host concatenates the per-core [8192, 2048]
slices. No collectives.

Per-core compute uses one level of Strassen: with A = x [N, K] and
B = w.T [K, O] split 2x2 (N/2=4096, K/2=2048, O/2=1024),

  M1=(A11+A22)(B11+B22)  M2=(A21+A22)B11  M3=A11(B12-B22)
  M4=A22(B21-B11)        M5=(A11+A12)B22  M6=(A21-A11)(B11+B12)
  M7=(A12-A22)(B21+B22)
  C11=M1+M4-M5+M7  C12=M3+M5  C21=M2+M4  C22=M1-M2+M3+M6

7/8 of the bf16 matmul cycles (PE floor 1747us -> 1529us). The B-side
combos (7 stationary-resident sets of 16 k-tiles x 512 cols, bf16) fit
112 KB/partition, so O is processed in 2 passes of 512 cols per
O-half; x streams twice. Weight sampling (softplus FMA) runs per
k-tile on ACT (Exp/Ln) + DVE and feeds the combo tiles, so the PE can
start as soon as k=0 is materialized. Tokens stream as pair-chunks
(128 from each N-half): 5 A-combos on DVE, 7 products into 7 PSUM
banks (ap=512 matmuls, 16-step K accumulation), then a DVE drain that
combines products into C quadrants in an order that frees PSUM banks
progressively, adds bias, and DMAs out on the GpSimd queue.
"""

import numpy as np
import ml_dtypes

import concourse.bass as bass
import concourse.mybir as mybir
import concourse.tile as tile
from concourse.bass_utils import run_bass_kernel_spmd
from concourse.vector_clock import ScopedClock, VectorClock

N_CORES = 8
N_TOK = 8192
IN_F = 4096
OUT_F = 16384
O_PER = OUT_F // N_CORES  # 2048 out features per core

P = 128
KT = 16                  # k-tiles per K-half (2048 / 128)
NCH = 32                 # token pair-chunks (each 128 + 128 tokens)
HALF_TOK = N_TOK // 2    # 4096
NPASS = 2                # O passes: 512 cols per O-half per pass
OC = 512

F32 = mybir.dt.float32
BF16 = mybir.dt.bfloat16
AF = mybir.ActivationFunctionType
ALU = mybir.AluOpType


def _patch_tile_drain():
    """The walrus build here caps sync-wait commands per CTRL_NO_STRUCT
    instruction; Tile's kernel-tail Drain overflows it. Spread the waits
    across nop carriers (one wait each) before the drain."""
    if getattr(tile.TileContext, "_drain_patched", False):
        return

    def _drain_and_barrier(self, tick_clock, wait_clock):
        nc = self.nc
        gc = tick_clock.global_clock
        n = len(gc)
        for i in range(n):
            t = gc[i]
            if t > 0:
                sub = [0] * n
                sub[i] = t
                carrier = nc.sync.nop(nofuse=True)
                wait_clock.add_sem_waits(
                    carrier.ins, ScopedClock({None: VectorClock(sub)})
                )
        nc.sync.drain()
        nc.all_engine_barrier()
        popped = nc._tile_sem_poison_stack.pop()
        assert popped is self._sem_poison
        nc.clear_and_free_semaphores(list(self.sems.allocated().values()))
        nc.all_engine_barrier()

    tile.TileContext._drain_and_barrier = _drain_and_barrier
    tile.TileContext._drain_patched = True


def _split_sync_waits(nc, max_waits=1):
    """This container's walrus build accepts at most ONE sync-wait command
    per instruction. Tile emits up to 3. Spill the excess onto same-engine
    InstNoOp carriers inserted immediately before the overloaded
    instruction."""
    n_spilled = 0
    for fn in nc.m.functions:
        for bb in fn.blocks:
            insts = list(bb.instructions)
            out = []
            changed = False
            for inst in insts:
                si = inst.sync_info
                if si is not None and si.on_wait and len(si.on_wait) > max_waits:
                    waits = list(si.on_wait)
                    spill, keep = waits[:-max_waits], waits[-max_waits:]
                    for w in spill:
                        nop = mybir.InstNoOp(
                            name=f"I-waitspill-{nc.next_id()}", ins=[], outs=[]
                        )
                        nop.engine = inst.engine
                        nop.sync_info = mybir.SyncInfo(on_wait=[w], on_update=[])
                        out.append(nop)
                        n_spilled += 1
                    inst.sync_info = mybir.SyncInfo(
                        on_wait=keep, on_update=list(si.on_update)
                    )
                    changed = True
                out.append(inst)
            if changed:
                bb.instructions = out
    return n_spilled


def _build():
    _patch_tile_drain()
    nc = bass.Bass()

    # x, pair-chunk-major: [64 chunks][128 part][32 kt][128 tok]; chunk
    # cp < 32 holds tokens cp*128.., cp >= 32 holds 4096 + (cp-32)*128..
    xp = nc.dram_tensor("xp", [2 * NCH, P, 2 * KT, P], BF16, kind="ExternalInput")
    # packed params: [pass][k][K-half][part][mu g0|g1, rho g0|g1, eps g0|g1]
    prm = nc.dram_tensor("prm", [NPASS, KT, 2, P, 3072], BF16, kind="ExternalInput")
    # bias params, pass-major column order, [1, 2048] each
    bmu = nc.dram_tensor("bmu", [1, O_PER], BF16, kind="ExternalInput")
    brho = nc.dram_tensor("brho", [1, O_PER], BF16, kind="ExternalInput")
    beps = nc.dram_tensor("beps", [1, O_PER], BF16, kind="ExternalInput")
    y = nc.dram_tensor("y", [N_TOK, O_PER], F32, kind="ExternalOutput")

    with tile.TileContext(nc) as tc:
        with (
            tc.tile_pool(name="bpool", bufs=1) as bpool,
            tc.tile_pool(name="xpool", bufs=2) as xpool,
            tc.tile_pool(name="apool", bufs=1) as apool,
            tc.tile_pool(name="opool", bufs=1) as opool,
            tc.tile_pool(name="spool", bufs=2) as spool,
            tc.tile_pool(name="fpool", bufs=1) as fpool,
            tc.tile_pool(name="qpool", bufs=2) as qpool,
            tc.tile_pool(name="biasp", bufs=1) as biasp,
            tc.tile_pool(name="psum", bufs=8, space="PSUM") as psump,
        ):
            # resident B-combo tiles: 7 combos x 16 k of [128, 512] bf16
            combo = {
                (i, k): bpool.tile([P, OC], BF16, name=f"c{i}_{k}", tag=f"c{i}_{k}")
                for i in range(1, 8)
                for k in range(KT)
            }
            bias_bc = biasp.tile([P, O_PER], BF16, name="bias_bc")

            # ── bias: softplus FMA on partition 0, then doubling ladder.
            # eps lands in bias_bc row 0; rho and mu ride stg-tag tiles.
            nc.sync.dma_start(bias_bc[0:1, :], beps[0:1, :])
            stg_r = spool.tile([P, 3072], BF16, name="stg", tag="stg")
            nc.sync.dma_start(stg_r[0:1, 0:O_PER], brho[0:1, :])
            for piece in range(2):
                sl = bass.ts(piece, 1024)
                fe = fpool.tile([P, 1024], F32, name="fexp", tag="fexp")
                fs = fpool.tile([P, 1024], F32, name="fsp", tag="fsp")
                nc.scalar.activation(fe[0:1, :], stg_r[0:1, sl], AF.Exp)
                nc.scalar.activation(fs[0:1, :], fe[0:1, :], AF.Ln, bias=1.0)
                nc.vector.tensor_mul(bias_bc[0:1, sl], fs[0:1, :], bias_bc[0:1, sl])
            stg_m = spool.tile([P, 3072], BF16, name="stg", tag="stg")
            nc.sync.dma_start(stg_m[0:1, 0:O_PER], bmu[0:1, :])
            nc.vector.tensor_add(
                bias_bc[0:1, :], bias_bc[0:1, :], stg_m[0:1, 0:O_PER]
            )
            rep = 1
            while rep < P:
                nc.sync.dma_start(bias_bc[rep : 2 * rep, :], bias_bc[0:rep, :])
                rep *= 2

            def materialize_k(h, k):
                # quadrant k-tiles: B11k -> combo2, B22k -> combo5,
                # B12k/B21k -> transient q tiles; then the 5 summed combos.
                qs = {}
                for half in range(2):
                    stg = spool.tile([P, 3072], BF16, name="stg", tag="stg")
                    nc.sync.dma_start(stg, prm[h, k, half])
                    fe = fpool.tile([P, 1024], F32, name="fexp", tag="fexp")
                    fs = fpool.tile([P, 1024], F32, name="fsp", tag="fsp")
                    nc.scalar.activation(fe, stg[:, 1024:2048], AF.Exp)
                    nc.scalar.activation(fs, fe, AF.Ln, bias=1.0)
                    nc.vector.tensor_mul(fe, fs, stg[:, 2048:3072])
                    if half == 0:
                        qa = qpool.tile([P, OC], BF16, name="qa", tag="qa")
                        d0, d1 = combo[(2, k)], qa
                        qs["B12"] = qa
                    else:
                        qb = qpool.tile([P, OC], BF16, name="qb", tag="qb")
                        d0, d1 = qb, combo[(5, k)]
                        qs["B21"] = qb
                    nc.vector.tensor_add(d0, fe[:, 0:OC], stg[:, 0:OC])
                    nc.vector.tensor_add(d1, fe[:, OC:1024], stg[:, OC:1024])
                b11, b22 = combo[(2, k)], combo[(5, k)]
                b12, b21 = qs["B12"], qs["B21"]
                nc.vector.tensor_add(combo[(1, k)], b11, b22)
                nc.vector.tensor_tensor(combo[(3, k)], b12, b22, ALU.subtract)
                nc.vector.tensor_tensor(combo[(4, k)], b21, b11, ALU.subtract)
                nc.vector.tensor_add(combo[(6, k)], b11, b12)
                nc.vector.tensor_add(combo[(7, k)], b21, b22)

            def do_chunk(h, c, kmajor):
                xlo = xpool.tile([P, 2 * KT, P], BF16, name="xlo", tag="xlo")
                nc.sync.dma_start(xlo, xp[c])
                xhi = xpool.tile([P, 2 * KT, P], BF16, name="xhi", tag="xhi")
                nc.sync.dma_start(xhi, xp[NCH + c])
                # A quadrants (transposed layout [K-part, kt, tok]):
                # A11 = xlo[:, :16], A12 = xlo[:, 16:], A21 = xhi[:, :16],
                # A22 = xhi[:, 16:]
                a = {}
                for i, (s0, s1, op) in {
                    1: (xlo[:, 0:KT, :], xhi[:, KT:, :], ALU.add),       # A11+A22
                    2: (xhi[:, 0:KT, :], xhi[:, KT:, :], ALU.add),       # A21+A22
                    5: (xlo[:, 0:KT, :], xlo[:, KT:, :], ALU.add),       # A11+A12
                    6: (xhi[:, 0:KT, :], xlo[:, 0:KT, :], ALU.subtract), # A21-A11
                    7: (xlo[:, KT:, :], xhi[:, KT:, :], ALU.subtract),   # A12-A22
                }.items():
                    a[i] = apool.tile([P, KT, P], BF16, name=f"a{i}", tag=f"a{i}")
                    nc.vector.tensor_tensor(a[i], s0, s1, op)

                def stat(i, k):
                    if i == 3:
                        return xlo[:, k, :]       # A11
                    if i == 4:
                        return xhi[:, KT + k, :]  # A22
                    return a[i][:, k, :]

                ps = {
                    i: psump.tile([P, OC], F32, name="ps", tag="ps")
                    for i in range(1, 8)
                }
                if kmajor:
                    for k in range(KT):
                        for i in range(1, 8):
                            nc.tensor.matmul(
                                ps[i], stat(i, k), combo[(i, k)],
                                start=(k == 0), stop=(k == KT - 1),
                            )
                else:
                    for i in range(1, 8):
                        for k in range(KT):
                            nc.tensor.matmul(
                                ps[i], stat(i, k), combo[(i, k)],
                                start=(k == 0), stop=(k == KT - 1),
                            )

                # drain: combine into C quadrants. DVE may read only ONE
                # PSUM operand per instruction, so chain stt ops (bias as
                # chain starter), ordered to free PSUM banks progressively.
                o11 = opool.tile([P, OC], F32, name="o11", tag="o11")
                o12 = opool.tile([P, OC], F32, name="o12", tag="o12")
                o21 = opool.tile([P, OC], F32, name="o21", tag="o21")
                o22 = opool.tile([P, OC], F32, name="o22", tag="o22")
                bs0 = bias_bc[:, h * 1024 : h * 1024 + OC]
                bs1 = bias_bc[:, h * 1024 + OC : (h + 1) * 1024]
                stt = nc.vector.scalar_tensor_tensor
                A, S, B_ = ALU.add, ALU.subtract, ALU.bypass
                stt(o11, ps[1], 1.0, bs0, op0=B_, op1=A)   # C11 = M1 + b0
                stt(o22, ps[1], 1.0, bs1, op0=B_, op1=A)   # C22 = M1 + b1
                stt(o21, ps[2], 1.0, bs0, op0=B_, op1=A)   # C21 = M2 + b0
                stt(o22, o22, 1.0, ps[2], op0=B_, op1=S)   # C22 -= M2
                stt(o12, ps[3], 1.0, bs1, op0=B_, op1=A)   # C12 = M3 + b1
                stt(o22, o22, 1.0, ps[3], op0=B_, op1=A)   # C22 += M3
                stt(o11, o11, 1.0, ps[4], op0=B_, op1=A)   # C11 += M4
                stt(o21, o21, 1.0, ps[4], op0=B_, op1=A)   # C21 += M4
                stt(o11, o11, 1.0, ps[5], op0=B_, op1=S)   # C11 -= M5
                stt(o12, o12, 1.0, ps[5], op0=B_, op1=A)   # C12 += M5
                stt(o22, o22, 1.0, ps[6], op0=B_, op1=A)   # C22 += M6
                stt(o11, o11, 1.0, ps[7], op0=B_, op1=A)   # C11 += M7
                t0 = c * P
                t1 = HALF_TOK + c * P
                g0 = slice(h * OC, (h + 1) * OC)
                g1 = slice(1024 + h * OC, 1024 + (h + 1) * OC)
                nc.gpsimd.dma_start(y[t0 : t0 + P, g0], o11)
                nc.gpsimd.dma_start(y[t0 : t0 + P, g1], o12)
                nc.gpsimd.dma_start(y[t1 : t1 + P, g0], o21)
                nc.gpsimd.dma_start(y[t1 : t1 + P, g1], o22)

            for h in range(NPASS):
                for k in range(KT):
                    materialize_k(h, k)
                for c in range(NCH):
                    do_chunk(h, c, kmajor=(h == 0 and c == 0))

    _split_sync_waits(nc)
    nc.finalize()
    return nc


_NC_CACHE = None


def _get_nc():
    global _NC_CACHE
    if _NC_CACHE is None:
        _NC_CACHE = _build()
    return _NC_CACHE


def prepare_in_maps(x, weight_mu, weight_rho, weight_eps, bias_mu, bias_rho, bias_eps):
    bf = ml_dtypes.bfloat16
    x = np.asarray(x, dtype=np.float32)
    weight_mu = np.asarray(weight_mu, dtype=np.float32)
    weight_rho = np.asarray(weight_rho, dtype=np.float32)
    weight_eps = np.asarray(weight_eps, dtype=np.float32)
    bias_mu = np.asarray(bias_mu, dtype=np.float32)
    bias_rho = np.asarray(bias_rho, dtype=np.float32)
    bias_eps = np.asarray(bias_eps, dtype=np.float32)

    # x packed pair-chunk-major: [64, 128, 32, 128]
    xT = np.ascontiguousarray(x.T).astype(bf)          # [4096, 8192]
    xr = xT.reshape(2 * KT, P, 2 * NCH, P)             # [kt, p, cp, tok]
    xp = np.ascontiguousarray(xr.transpose(2, 1, 0, 3))  # [cp, p, kt, tok]

    # pass-major bias column order
    bcols = np.r_[0:512, 1024:1536, 512:1024, 1536:2048]

    in_maps = []
    for co in range(N_CORES):
        osl = slice(co * O_PER, (co + 1) * O_PER)
        # [3, 4096, 2048] = (mu, rho, eps) x [K, O_core]
        wt = np.stack(
            [
                weight_mu[osl, :].T,
                weight_rho[osl, :].T,
                weight_eps[osl, :].T,
            ]
        ).astype(bf)
        # rows -> [3, half, k, p, O]; cols per pass h: g0, g1
        wr = wt.reshape(3, 2, KT, P, O_PER)
        prm = np.empty((NPASS, KT, 2, P, 3072), dtype=bf)
        for h in range(NPASS):
            cols = np.r_[h * OC : (h + 1) * OC, 1024 + h * OC : 1024 + (h + 1) * OC]
            sel = wr[:, :, :, :, cols]                # [3, half, k, p, 1024]
            prm[h] = np.ascontiguousarray(
                sel.transpose(2, 1, 3, 0, 4)          # [k, half, p, 3, 1024]
            ).reshape(KT, 2, P, 3072)
        in_maps.append(
            {
                "xp": xp,
                "prm": prm,
                "bmu": bias_mu[osl][bcols].reshape(1, O_PER).astype(bf),
                "brho": bias_rho[osl][bcols].reshape(1, O_PER).astype(bf),
                "beps": bias_eps[osl][bcols].reshape(1, O_PER).astype(bf),
            }
        )
    return in_maps


def run(in_maps, trace=False):
    nc = _get_nc()
    res = run_bass_kernel_spmd(nc, in_maps, list(range(N_CORES)), trace=trace)
    out = np.concatenate([res.results[c]["y"] for c in range(N_CORES)], axis=1)
    return out, res


def kernel(**inputs) -> np.ndarray:
    in_maps = prepare_in_maps(**inputs)
    out, _ = run(in_maps, trace=False)
    return out
